# revision 9
# baseline (speedup 1.0000x reference)
"""GAT (graph-attention) message-passing kernel for Trainium2, 8 NeuronCores.

Strategy (dst-sharded, degree-sorted edge grid):
  - Each core owns N/8 destination nodes and all their in-edges.
  - Host-side index preprocessing (graph partitioning per the sharding hint):
    dst nodes are relabeled by descending in-degree so each 128-node block
    has a uniform max degree; edges are laid out in a [dst, degree-slot]
    grid.  Pad slots point at a crafted table row whose attention logit is
    -1e9, so exp() underflows to exactly 0 and pads contribute nothing.
  - Each core receives a compacted-src copy of x (own nodes first, then halo
    sources).  The hardware dma_gather uses int16 indices (<32768), but the
    per-core table is ~49k rows, so every block's degree axis is split into a
    "lo" range (table rows < 32768, window base 0) and a "hi" range (window
    based so it ends at a duplicated pad row).  Two gathers per superblock.
  - On device: one fused projection matmul produces feat and er (via a
    folded W@attn_r column block); feat lands in a DRAM table; per
    superblock dma_gather pulls feat[src] for the grid; VectorE/ACT compute
    scores, softmax (no max-subtraction: logits are O(1)), weighted messages
    and degree-axis reductions; normalization by the softmax denominator
    happens after aggregation (rst/s), then bias + head-mean.
"""

import sys

if "/opt/trn_rl_repo" not in sys.path:
    sys.path.insert(0, "/opt/trn_rl_repo")

import numpy as np
import ml_dtypes

import concourse.bacc as bacc
import concourse.mybir as mybir
import concourse.tile as tile
from concourse.bass_utils import run_bass_kernel_spmd

P = 128
LOW = 32768  # int16 index window
FP32 = mybir.dt.float32
BF16 = mybir.dt.bfloat16
I16 = mybir.dt.int16

DEFAULT_CFG = dict(
    N=50000,
    E=1600000,
    IN=256,
    H=8,
    F=16,
    C=8,
    SUPER=2,
    neg_slope=0.2,
    table_bf16=True,
)


def _wrap_idx(a):
    """[n] int -> [128, n//16] int16, hardware dma_gather wrapped layout."""
    n = a.size
    assert n % 16 == 0
    w = a.reshape(n // 16, 16).T.astype(np.int16)
    return np.ascontiguousarray(np.tile(w, (8, 1)))


def _grid_fill(nl_s, rows_s, off, Nc, NB, GT, pad_idx):
    """Slot-grid index array for one (already nl-sorted) edge subset."""
    starts = np.searchsorted(nl_s, np.arange(Nc))
    rank = np.arange(nl_s.size) - starts[nl_s]
    b = nl_s // P
    p = nl_s % P
    slot = (off[b] + rank) * P + p
    gidx = np.full(GT * P, pad_idx, np.int64)
    gidx[slot] = rows_s
    return gidx


def preprocess(x, W, attn_l, attn_r, bias, src, dst, cfg):
    N, IN, H, F, C = cfg["N"], cfg["IN"], cfg["H"], cfg["F"], cfg["C"]
    Nc = N // C
    HF = H * F
    NB = (Nc + P - 1) // P

    x = np.asarray(x, np.float32)
    W = np.asarray(W, np.float32)
    attn_l = np.asarray(attn_l, np.float32)
    attn_r = np.asarray(attn_r, np.float32)
    bias = np.asarray(bias, np.float32)
    src = np.asarray(src).astype(np.int64)
    dst = np.asarray(dst).astype(np.int64)

    tdt = ml_dtypes.bfloat16 if cfg["table_bf16"] else np.float32

    owner = dst // Nc
    cores = []
    KMAX = 0
    for c in range(C):
        m = owner == c
        es = src[m]
        dl = dst[m] - c * Nc
        deg = np.bincount(dl, minlength=Nc)
        order = np.argsort(-deg, kind="stable")  # new local id -> old local id
        newid = np.empty(Nc, np.int64)
        newid[order] = np.arange(Nc)
        nl = newid[dl]
        own_glob = c * Nc + order
        others = np.setdiff1d(np.unique(es), own_glob)
        comp_nodes = np.concatenate([own_glob, others])
        K = comp_nodes.size
        KMAX = max(KMAX, K)
        cores.append(dict(es=es, nl=nl, order=order, comp_nodes=comp_nodes, K=K))

    KP2 = ((KMAX + P - 1) // P) * P
    TROWS = P + KP2 + 1  # [pad block 128 | comp KP2 | pad copy]
    assert TROWS <= 2 * LOW, f"table too large: {TROWS}"
    BHI = max(TROWS - LOW, 0)
    pad_hi = TROWS - 1 - BHI

    # per-core lo/hi degree profiles, nl-sorted edge arrays
    for co in cores:
        compid = np.full(N, -1, np.int64)
        compid[co["comp_nodes"]] = np.arange(co["K"])
        sidx = np.argsort(co["nl"], kind="stable")
        nl_s = co["nl"][sidx]
        rows_s = P + compid[co["es"][sidx]]
        assert (rows_s >= P).all() and (rows_s < TROWS - 1).all()
        is_lo = rows_s < LOW
        co["nl_lo"], co["rows_lo"] = nl_s[is_lo], rows_s[is_lo]
        co["nl_hi"], co["rows_hi"] = nl_s[~is_lo], rows_s[~is_lo] - BHI
        dlo = np.zeros(NB * P, np.int64)
        dlo[:Nc] = np.bincount(co["nl_lo"], minlength=Nc)
        dhi = np.zeros(NB * P, np.int64)
        dhi[:Nc] = np.bincount(co["nl_hi"], minlength=Nc)
        co["dlo_b"] = dlo.reshape(NB, P).max(axis=1)
        co["dhi_b"] = dhi.reshape(NB, P).max(axis=1)

    DLO = np.maximum(np.max(np.stack([co["dlo_b"] for co in cores]), axis=0), 1)
    DHI = np.max(np.stack([co["dhi_b"] for co in cores]), axis=0)
    off_lo = np.zeros(NB + 1, np.int64)
    off_lo[1:] = np.cumsum(DLO)
    off_hi = np.zeros(NB + 1, np.int64)
    off_hi[1:] = np.cumsum(DHI)
    GT_lo, GT_hi = int(off_lo[-1]), int(off_hi[-1])

    # folded params
    W_er = np.einsum("khf,hf->kh", W.reshape(IN, H, F), attn_r)
    W_cat = np.ascontiguousarray(np.concatenate([W, W_er], axis=1)).astype(tdt)
    attnl_rep = np.tile(attn_l.reshape(1, HF), (P, 1)).astype(tdt)
    bias_rep = np.ascontiguousarray(
        np.tile(bias.reshape(H, F).mean(axis=0, keepdims=True), (P, 1))
    ).astype(np.float32)
    v = np.zeros(HF, np.float32)
    for h in range(H):
        a = attn_l[h]
        v[h * F : (h + 1) * F] = -1e9 * a / max(float((a * a).sum()), 1e-12)
    chk = (v.reshape(H, F).astype(tdt).astype(np.float32) * attn_l).sum(axis=1)
    assert (chk < -1e8).all(), f"pad row logits not very negative: {chk}"
    pad_rep = np.ascontiguousarray(np.tile(v.reshape(1, HF), (P, 1))).astype(tdt)

    in_maps = []
    for c, co in enumerate(cores):
        gl = _grid_fill(co["nl_lo"], co["rows_lo"], off_lo, Nc, NB, GT_lo, 0)
        assert gl.max() < LOW
        widx_lo = _wrap_idx(gl)
        if GT_hi > 0:
            gh = _grid_fill(co["nl_hi"], co["rows_hi"], off_hi, Nc, NB, GT_hi, pad_hi)
            assert gh.max() < LOW and gh.min() >= 0
            widx_hi = _wrap_idx(gh)
        else:
            widx_hi = np.zeros((P, 16), np.int16)
        xc = np.zeros((KP2, IN), np.float32)
        xc[: co["K"]] = x[co["comp_nodes"]]
        xcT = np.ascontiguousarray(xc.T).astype(tdt)
        in_maps.append(
            dict(
                xcT=xcT,
                wcat=W_cat,
                attnl=attnl_rep,
                biasm=bias_rep,
                padrow=pad_rep,
                widx_lo=widx_lo,
                widx_hi=widx_hi,
            )
        )

    params = dict(
        KP2=KP2,
        TROWS=TROWS,
        BHI=BHI,
        NB=NB,
        Nc=Nc,
        GT_lo=GT_lo,
        GT_hi=GT_hi,
        DLO=[int(d) for d in DLO],
        DHI=[int(d) for d in DHI],
        off_lo=[int(o) for o in off_lo],
        off_hi=[int(o) for o in off_hi],
    )
    orders = [co["order"] for co in cores]
    return in_maps, params, orders


def build_program(cfg, params):
    IN, H, F, C = cfg["IN"], cfg["H"], cfg["F"], cfg["C"]
    HF = H * F
    KP2, TROWS, BHI, NB = (
        params["KP2"],
        params["TROWS"],
        params["BHI"],
        params["NB"],
    )
    GT_lo, GT_hi = params["GT_lo"], params["GT_hi"]
    DLO, DHI = params["DLO"], params["DHI"]
    off_lo, off_hi = params["off_lo"], params["off_hi"]
    SUPER = cfg["SUPER"]
    phase = {1: 1, 21: 2, 22: 3, 2: 4, 3: 5, 4: 6}[cfg.get("phase", 4)]
    tdt = BF16 if cfg["table_bf16"] else FP32
    slope = float(cfg["neg_slope"])
    KC = IN // P
    MT = KP2 // P
    DMAXB = max(DLO[b] + DHI[b] for b in range(NB))
    sbs = list(range(0, NB, SUPER))
    MAXG = max(
        (off_lo[min(s + SUPER, NB)] - off_lo[s])
        + (off_hi[min(s + SUPER, NB)] - off_hi[s])
        for s in sbs
    )

    nc = bacc.Bacc("TRN2", target_bir_lowering=False, debug=False, num_devices=C)
    xcT_d = nc.dram_tensor("xcT", [IN, KP2], tdt, kind="ExternalInput").ap()
    wcat_d = nc.dram_tensor("wcat", [IN, HF + H], tdt, kind="ExternalInput").ap()
    attnl_d = nc.dram_tensor("attnl", [P, HF], tdt, kind="ExternalInput").ap()
    biasm_d = nc.dram_tensor("biasm", [P, F], FP32, kind="ExternalInput").ap()
    padrow_d = nc.dram_tensor("padrow", [P, HF], tdt, kind="ExternalInput").ap()
    widx_lo_d = nc.dram_tensor(
        "widx_lo", [P, 8 * GT_lo], I16, kind="ExternalInput"
    ).ap()
    widx_hi_d = nc.dram_tensor(
        "widx_hi", [P, 8 * max(GT_hi, 2)], I16, kind="ExternalInput"
    ).ap()
    out_d = nc.dram_tensor("out", [NB * P, F], FP32, kind="ExternalOutput").ap()

    with tile.TileContext(nc) as tc:
        with (
            tc.tile_pool(name="dram", bufs=1, space="DRAM") as dpool,
            tc.tile_pool(name="const", bufs=1) as cpool,
            tc.tile_pool(name="xa", bufs=3) as xpool,
            tc.tile_pool(name="proj", bufs=4) as fpool,
            tc.tile_pool(name="psum", bufs=4, space="PSUM") as ppool,
            tc.tile_pool(name="gat", bufs=2) as gpool,
            tc.tile_pool(name="widx", bufs=2) as wpool,
            tc.tile_pool(name="blk", bufs=3) as bpool,
        ):
            feat = dpool.tile([TROWS, HF], tdt)

            # constants
            w_t = []
            for k in range(KC):
                wt = cpool.tile([P, HF + H], tdt, tag=f"w{k}")
                nc.sync.dma_start(out=wt[:], in_=wcat_d[k * P : (k + 1) * P, :])
                w_t.append(wt)
            at = cpool.tile([P, HF], tdt, tag="attnl")
            nc.sync.dma_start(out=at[:], in_=attnl_d[:])
            bi = cpool.tile([P, F], FP32, tag="biasm")
            nc.sync.dma_start(out=bi[:], in_=biasm_d[:])
            er_all = cpool.tile([P, NB * H], FP32, tag="erall")

            # pad rows -> feat[0:128] and feat[TROWS-1]
            prt = cpool.tile([P, HF], tdt, tag="padrow")
            nc.sync.dma_start(out=prt[:], in_=padrow_d[:])
            nc.sync.dma_start(out=feat[0:P, :], in_=prt[:])
            nc.sync.dma_start(out=feat[TROWS - 1 : TROWS, :], in_=prt[0:1, :])

            # projection: feat[128+m*128 ...] = xcT[:, m].T @ [W | W@attn_r]
            XB = 4
            for m0 in range(0, MT, XB):
                nb = min(XB, MT - m0)
                xts = []
                for k in range(KC):
                    xa = xpool.tile([P, XB * P], tdt, tag=f"xa{k}")
                    nc.sync.dma_start(
                        out=xa[:, : nb * P],
                        in_=xcT_d[k * P : (k + 1) * P, m0 * P : (m0 + nb) * P],
                    )
                    xts.append(xa)
                for j in range(nb):
                    m = m0 + j
                    fsb = fpool.tile([P, HF], tdt, tag="fsb")
                    if cfg.get("nomm"):
                        nc.vector.tensor_copy(
                            out=fsb[:], in_=xts[0][:, j * P : (j + 1) * P]
                        )
                        pt = None
                    else:
                        pt = ppool.tile([P, HF + H], FP32, space="PSUM")
                        for k in range(KC):
                            nc.tensor.matmul(
                                out=pt[:],
                                lhsT=xts[k][:, j * P : (j + 1) * P],
                                rhs=w_t[k][:],
                                start=(k == 0),
                                stop=(k == KC - 1),
                            )
                        nc.vector.tensor_copy(out=fsb[:], in_=pt[:, :HF])
                    nc.sync.dma_start(
                        out=feat[P + m * P : P + (m + 1) * P, :], in_=fsb[:]
                    )
                    if m < NB and pt is not None:
                        nc.vector.tensor_copy(
                            out=er_all[:, m * H : (m + 1) * H],
                            in_=pt[:, HF : HF + H],
                        )

            # edge phase
            for s0 in (sbs if phase >= 2 else []):  # noqa
                s1 = min(s0 + SUPER, NB)
                glo = off_lo[s1] - off_lo[s0]
                ghi = off_hi[s1] - off_hi[s0]
                clo = off_lo[s0]
                chi = off_hi[s0]
                wtl = wpool.tile([P, 8 * MAXG], I16, tag="widx")
                nc.sync.dma_start(
                    out=wtl[:, : 8 * glo],
                    in_=widx_lo_d[:, 8 * clo : 8 * (clo + glo)],
                )
                if ghi > 0:
                    nc.sync.dma_start(
                        out=wtl[:, 8 * glo : 8 * (glo + ghi)],
                        in_=widx_hi_d[:, 8 * chi : 8 * (chi + ghi)],
                    )
                G = gpool.tile([P, MAXG * HF], tdt, tag="G")
                nc.gpsimd.dma_gather(
                    G[:, : glo * HF].rearrange("p (g e) -> p g e", e=HF),
                    feat[0:LOW, :] if TROWS > LOW else feat[:],
                    wtl[:, : 8 * glo],
                    P * glo,
                    P * glo,
                    HF,
                    single_packet=False,
                )
                if ghi > 0:
                    nc.gpsimd.dma_gather(
                        G[:, glo * HF : (glo + ghi) * HF].rearrange(
                            "p (g e) -> p g e", e=HF
                        ),
                        feat[BHI:TROWS, :],
                        wtl[:, 8 * glo : 8 * (glo + ghi)],
                        P * ghi,
                        P * ghi,
                        HF,
                        single_packet=False,
                    )
                for b in range(s0, s1):
                    DL, DH = DLO[b], DHI[b]
                    D = DL + DH
                    pieces = [(G[:, (off_lo[b] - clo) * HF :], DL, 0)]
                    if DH > 0:
                        pieces.append(
                            (G[:, (glo + off_hi[b] - chi) * HF :], DH, DL)
                        )
                    elp = bpool.tile([P, DMAXB * HF], tdt, tag="elp")
                    sc = bpool.tile([P, DMAXB * H], FP32, tag="sc")
                    for gsl, dn, d0 in (pieces if phase >= 5 else []):
                        # el = sum_f feat[src]*attn_l
                        nc.vector.tensor_tensor(
                            out=elp[:, d0 * HF : (d0 + dn) * HF].rearrange(
                                "p (g e) -> p g e", e=HF
                            ),
                            in0=gsl[:, : dn * HF].rearrange(
                                "p (g e) -> p g e", e=HF
                            ),
                            in1=at[:]
                            .rearrange("p (o e) -> p o e", o=1)
                            .to_broadcast([P, dn, HF]),
                            op=mybir.AluOpType.mult,
                        )
                        nc.vector.reduce_sum(
                            out=sc[:, d0 * H : (d0 + dn) * H].rearrange(
                                "p (g h) -> p g h", h=H
                            ),
                            in_=elp[:, d0 * HF : (d0 + dn) * HF].rearrange(
                                "p (g h f) -> p g h f", h=H, f=F
                            ),
                            axis=mybir.AxisListType.X,
                        )
                    # scores = leaky(el + er); ex = exp
                    if phase < 5:
                        o_t = bpool.tile([P, F], FP32, tag="o")
                        if phase == 2:
                            nc.vector.tensor_copy(
                                out=o_t[:], in_=pieces[0][0][:, :F]
                            )
                            nc.sync.dma_start(
                                out=out_d[b * P : (b + 1) * P, :], in_=o_t[:]
                            )
                            continue
                        rst = bpool.tile([P, HF], FP32, tag="rst")
                        rst2 = bpool.tile([P, HF], FP32, tag="rst2")
                        for i, (gsl, dn, d0) in enumerate(pieces):
                            nc.vector.reduce_sum(
                                out=(rst if i == 0 else rst2)[:],
                                in_=gsl[:, : dn * HF].rearrange(
                                    "p (g e) -> p e g", e=HF
                                ),
                                axis=mybir.AxisListType.X,
                            )
                        if len(pieces) > 1:
                            nc.vector.tensor_add(out=rst[:], in0=rst[:], in1=rst2[:])
                        if phase == 3:
                            nc.vector.tensor_copy(out=o_t[:], in_=rst[:, :F])
                        else:
                            nc.vector.reduce_sum(
                                out=o_t[:],
                                in_=rst[:].rearrange("p (h f) -> p f h", h=H),
                                axis=mybir.AxisListType.X,
                            )
                        nc.sync.dma_start(
                            out=out_d[b * P : (b + 1) * P, :], in_=o_t[:]
                        )
                        continue
                    nc.vector.tensor_tensor(
                        out=sc[:, : D * H].rearrange("p (g h) -> p g h", h=H),
                        in0=sc[:, : D * H].rearrange("p (g h) -> p g h", h=H),
                        in1=er_all[:, b * H : (b + 1) * H]
                        .rearrange("p (o h) -> p o h", o=1)
                        .to_broadcast([P, D, H]),
                        op=mybir.AluOpType.add,
                    )
                    nc.vector.scalar_tensor_tensor(
                        out=sc[:, : D * H],
                        in0=sc[:, : D * H],
                        scalar=slope,
                        in1=sc[:, : D * H],
                        op0=mybir.AluOpType.mult,
                        op1=mybir.AluOpType.max,
                    )
                    ex = bpool.tile([P, DMAXB * H], tdt, tag="ex")
                    nc.scalar.activation(
                        out=ex[:, : D * H],
                        in_=sc[:, : D * H],
                        func=mybir.ActivationFunctionType.Exp,
                    )
                    # s = sum_deg ex ; r = 1/max(s,eps)
                    s_t = bpool.tile([P, H], FP32, tag="s")
                    nc.vector.reduce_sum(
                        out=s_t[:],
                        in_=ex[:, : D * H].rearrange("p (g h) -> p h g", h=H),
                        axis=mybir.AxisListType.X,
                    )
                    r_t = bpool.tile([P, H], FP32, tag="r")
                    nc.vector.tensor_scalar_max(out=r_t[:], in0=s_t[:], scalar1=1e-9)
                    nc.vector.reciprocal(out=r_t[:], in_=r_t[:])
                    # weighted messages (in place on G) + aggregation
                    rst = bpool.tile([P, HF], FP32, tag="rst")
                    rst2 = bpool.tile([P, HF], FP32, tag="rst2")
                    if phase < 6:
                        for i, (gsl, dn, d0) in enumerate(pieces):
                            nc.vector.reduce_sum(
                                out=(rst if i == 0 else rst2)[:],
                                in_=gsl[:, : dn * HF].rearrange(
                                    "p (g e) -> p e g", e=HF
                                ),
                                axis=mybir.AxisListType.X,
                            )
                        if len(pieces) > 1:
                            nc.vector.tensor_add(out=rst[:], in0=rst[:], in1=rst2[:])
                        o_t = bpool.tile([P, F], FP32, tag="o")
                        nc.vector.reduce_sum(
                            out=o_t[:],
                            in_=rst[:].rearrange("p (h f) -> p f h", h=H),
                            axis=mybir.AxisListType.X,
                        )
                        nc.sync.dma_start(
                            out=out_d[b * P : (b + 1) * P, :], in_=o_t[:]
                        )
                        continue
                    for i, (gsl, dn, d0) in enumerate(pieces):
                        nc.vector.tensor_tensor(
                            out=gsl[:, : dn * HF].rearrange(
                                "p (g h f) -> p g h f", h=H, f=F
                            ),
                            in0=gsl[:, : dn * HF].rearrange(
                                "p (g h f) -> p g h f", h=H, f=F
                            ),
                            in1=ex[:, d0 * H : (d0 + dn) * H]
                            .rearrange("p (g h o) -> p g h o", h=H, o=1)
                            .to_broadcast([P, dn, H, F]),
                            op=mybir.AluOpType.mult,
                        )
                        nc.vector.reduce_sum(
                            out=(rst if i == 0 else rst2)[:],
                            in_=gsl[:, : dn * HF].rearrange(
                                "p (g e) -> p e g", e=HF
                            ),
                            axis=mybir.AxisListType.X,
                        )
                    if len(pieces) > 1:
                        nc.vector.tensor_add(out=rst[:], in0=rst[:], in1=rst2[:])
                    # normalize, head-mean, bias
                    nc.vector.tensor_tensor(
                        out=rst[:].rearrange("p (h f) -> p h f", h=H),
                        in0=rst[:].rearrange("p (h f) -> p h f", h=H),
                        in1=r_t[:]
                        .rearrange("p (h o) -> p h o", o=1)
                        .to_broadcast([P, H, F]),
                        op=mybir.AluOpType.mult,
                    )
                    o_t = bpool.tile([P, F], FP32, tag="o")
                    nc.vector.reduce_sum(
                        out=o_t[:],
                        in_=rst[:].rearrange("p (h f) -> p f h", h=H),
                        axis=mybir.AxisListType.X,
                    )
                    nc.vector.scalar_tensor_tensor(
                        out=o_t[:],
                        in0=o_t[:],
                        scalar=1.0 / H,
                        in1=bi[:],
                        op0=mybir.AluOpType.mult,
                        op1=mybir.AluOpType.add,
                    )
                    nc.sync.dma_start(out=out_d[b * P : (b + 1) * P, :], in_=o_t[:])

    nc.compile()
    return nc


def _run(inputs, cfg, trace=False):
    in_maps, params, orders = preprocess(
        inputs["x"],
        inputs["W"],
        inputs["attn_l"],
        inputs["attn_r"],
        inputs["bias"],
        inputs["src"],
        inputs["dst"],
        cfg,
    )
    nc = build_program(cfg, params)
    res = run_bass_kernel_spmd(nc, in_maps, list(range(cfg["C"])), trace=trace)
    N, C, F = cfg["N"], cfg["C"], cfg["F"]
    Nc = N // C
    out = np.empty((N, F), np.float32)
    for c in range(C):
        oc = res.results[c]["out"]
        out[c * Nc + orders[c]] = oc[:Nc]
    return out, res


def kernel(**inputs):
    out, _ = _run(inputs, DEFAULT_CFG, trace=False)
    return out


# revision 13
# speedup vs baseline: 1.2988x; 1.2988x over previous
"""GAT (graph-attention) message-passing kernel for Trainium2, 8 NeuronCores.

Strategy (dst-sharded, degree-sorted edge grid):
  - Each core owns N/8 destination nodes and all their in-edges.
  - Host-side index preprocessing (graph partitioning per the sharding hint):
    dst nodes are relabeled by descending in-degree so each 128-node block
    has a uniform max degree; edges are laid out in a [dst, degree-slot]
    grid.  Pad slots point at a crafted table row whose attention logit is
    -1e9, so exp() underflows to exactly 0 and pads contribute nothing.
  - Each core receives a compacted-src copy of x (own nodes first, then halo
    sources).  The hardware dma_gather uses int16 indices (<32768), but the
    per-core table is ~49k rows, so every block's degree axis is split into a
    "lo" range (table rows < 32768, window base 0) and a "hi" range (window
    based so it ends at a duplicated pad row).  Two gathers per superblock.
  - On device: one fused projection matmul produces feat and er (via a
    folded W@attn_r column block); feat lands in a DRAM table; per
    superblock dma_gather pulls feat[src] for the grid; VectorE/ACT compute
    scores, softmax (no max-subtraction: logits are O(1)), weighted messages
    and degree-axis reductions; normalization by the softmax denominator
    happens after aggregation (rst/s), then bias + head-mean.
"""

import sys

if "/opt/trn_rl_repo" not in sys.path:
    sys.path.insert(0, "/opt/trn_rl_repo")

import numpy as np
import ml_dtypes

import concourse.bacc as bacc
import concourse.mybir as mybir
import concourse.tile as tile
from concourse.bass_utils import run_bass_kernel_spmd

P = 128
LOW = 32768  # int16 index window
FP32 = mybir.dt.float32
BF16 = mybir.dt.bfloat16
I16 = mybir.dt.int16

DEFAULT_CFG = dict(
    N=50000,
    E=1600000,
    IN=256,
    H=8,
    F=16,
    C=8,
    SUPER=1,
    neg_slope=0.2,
    table_bf16=True,
)


def _wrap_idx(a):
    """[n] int -> [128, n//16] int16, hardware dma_gather wrapped layout."""
    n = a.size
    assert n % 16 == 0
    w = a.reshape(n // 16, 16).T.astype(np.int16)
    return np.ascontiguousarray(np.tile(w, (8, 1)))


def _grid_fill(nl_s, rows_s, off, Nc, NB, GT, pad_idx):
    """Slot-grid index array for one (already nl-sorted) edge subset."""
    starts = np.searchsorted(nl_s, np.arange(Nc))
    rank = np.arange(nl_s.size) - starts[nl_s]
    b = nl_s // P
    p = nl_s % P
    slot = (off[b] + rank) * P + p
    gidx = np.full(GT * P, pad_idx, np.int64)
    gidx[slot] = rows_s
    return gidx


def preprocess(x, W, attn_l, attn_r, bias, src, dst, cfg):
    N, IN, H, F, C = cfg["N"], cfg["IN"], cfg["H"], cfg["F"], cfg["C"]
    Nc = N // C
    HF = H * F
    NB = (Nc + P - 1) // P

    x = np.asarray(x, np.float32)
    W = np.asarray(W, np.float32)
    attn_l = np.asarray(attn_l, np.float32)
    attn_r = np.asarray(attn_r, np.float32)
    bias = np.asarray(bias, np.float32)
    src = np.asarray(src).astype(np.int64)
    dst = np.asarray(dst).astype(np.int64)

    tdt = ml_dtypes.bfloat16 if cfg["table_bf16"] else np.float32

    # degree-balanced dst->core assignment: rank nodes by global in-degree;
    # core = rank % C, new local id = rank // C  (so every core sees an
    # almost identical descending-degree profile and cross-core max padding
    # vanishes).  orders[c][nl] = global node owned by core c at local nl.
    gdeg = np.bincount(dst, minlength=N)
    grank = np.argsort(-gdeg, kind="stable")  # rank -> global node
    node_core = np.empty(N, np.int64)
    node_nl = np.empty(N, np.int64)
    ranks = np.arange(N)
    node_core[grank] = ranks % C
    node_nl[grank] = ranks // C
    cores = []
    KMAX = 0
    for c in range(C):
        m = node_core[dst] == c
        es = src[m]
        ed = dst[m]
        own_glob = grank[np.arange(Nc) * C + c]
        others = np.setdiff1d(np.unique(es), own_glob)
        # lo membership is order-independent: own region occupies table rows
        # [P, P+Nc) (always < LOW); 'others' rows follow in sorted order.
        n_lo_others = max(0, min(LOW - P - Nc, others.size))
        in_lo = np.zeros(N, bool)
        in_lo[own_glob] = True
        in_lo[others[:n_lo_others]] = True
        deg_lo_g = np.bincount(ed[in_lo[es]], minlength=N)
        deg_hi_g = np.bincount(ed[~in_lo[es]], minlength=N)
        dlo_own = deg_lo_g[own_glob]
        dhi_own = deg_hi_g[own_glob]
        # dst order: deg_lo-major, deg_hi-minor (descending) -> blocks are
        # homogeneous in both lo and hi slot counts.
        perm = np.lexsort((-dhi_own, -dlo_own))
        order = own_glob[perm]
        newid = np.full(N, -1, np.int64)
        newid[order] = np.arange(Nc)
        nl = newid[ed]
        comp_nodes = np.concatenate([order, others])
        K = comp_nodes.size
        KMAX = max(KMAX, K)
        cores.append(dict(es=es, nl=nl, order=order, comp_nodes=comp_nodes, K=K))

    KP2 = ((KMAX + P - 1) // P) * P
    TROWS = P + KP2 + 1  # [pad block 128 | comp KP2 | pad copy]
    assert TROWS <= 2 * LOW, f"table too large: {TROWS}"
    BHI = max(TROWS - LOW, 0)
    pad_hi = TROWS - 1 - BHI

    # per-core lo/hi degree profiles, nl-sorted edge arrays
    for co in cores:
        compid = np.full(N, -1, np.int64)
        compid[co["comp_nodes"]] = np.arange(co["K"])
        sidx = np.argsort(co["nl"], kind="stable")
        nl_s = co["nl"][sidx]
        rows_s = P + compid[co["es"][sidx]]
        assert (rows_s >= P).all() and (rows_s < TROWS - 1).all()
        is_lo = rows_s < LOW
        co["nl_lo"], co["rows_lo"] = nl_s[is_lo], rows_s[is_lo]
        co["nl_hi"], co["rows_hi"] = nl_s[~is_lo], rows_s[~is_lo] - BHI
        dlo = np.zeros(NB * P, np.int64)
        dlo[:Nc] = np.bincount(co["nl_lo"], minlength=Nc)
        dhi = np.zeros(NB * P, np.int64)
        dhi[:Nc] = np.bincount(co["nl_hi"], minlength=Nc)
        co["dlo_b"] = dlo.reshape(NB, P).max(axis=1)
        co["dhi_b"] = dhi.reshape(NB, P).max(axis=1)

    DLO = np.maximum(np.max(np.stack([co["dlo_b"] for co in cores]), axis=0), 1)
    DHI = np.max(np.stack([co["dhi_b"] for co in cores]), axis=0)
    off_lo = np.zeros(NB + 1, np.int64)
    off_lo[1:] = np.cumsum(DLO)
    off_hi = np.zeros(NB + 1, np.int64)
    off_hi[1:] = np.cumsum(DHI)
    GT_lo, GT_hi = int(off_lo[-1]), int(off_hi[-1])

    # folded params
    W_er = np.einsum("khf,hf->kh", W.reshape(IN, H, F), attn_r)
    W_cat = np.ascontiguousarray(np.concatenate([W, W_er], axis=1)).astype(tdt)
    attnl_rep = np.tile(attn_l.reshape(1, HF), (P, 1)).astype(tdt)
    bias_rep = np.ascontiguousarray(
        np.tile(bias.reshape(H, F).mean(axis=0, keepdims=True), (P, 1))
    ).astype(np.float32)
    v = np.zeros(HF, np.float32)
    for h in range(H):
        a = attn_l[h]
        v[h * F : (h + 1) * F] = -1e9 * a / max(float((a * a).sum()), 1e-12)
    chk = (v.reshape(H, F).astype(tdt).astype(np.float32) * attn_l).sum(axis=1)
    assert (chk < -1e8).all(), f"pad row logits not very negative: {chk}"
    pad_rep = np.ascontiguousarray(np.tile(v.reshape(1, HF), (P, 1))).astype(tdt)

    in_maps = []
    for c, co in enumerate(cores):
        gl = _grid_fill(co["nl_lo"], co["rows_lo"], off_lo, Nc, NB, GT_lo, 0)
        assert gl.max() < LOW
        widx_lo = _wrap_idx(gl)
        if GT_hi > 0:
            gh = _grid_fill(co["nl_hi"], co["rows_hi"], off_hi, Nc, NB, GT_hi, pad_hi)
            assert gh.max() < LOW and gh.min() >= 0
            widx_hi = _wrap_idx(gh)
        else:
            widx_hi = np.zeros((P, 16), np.int16)
        xc = np.zeros((KP2, IN), np.float32)
        xc[: co["K"]] = x[co["comp_nodes"]]
        xcT = np.ascontiguousarray(xc.T).astype(tdt)
        in_maps.append(
            dict(
                xcT=xcT,
                wcat=W_cat,
                attnl=attnl_rep,
                biasm=bias_rep,
                padrow=pad_rep,
                widx_lo=widx_lo,
                widx_hi=widx_hi,
            )
        )

    params = dict(
        KP2=KP2,
        TROWS=TROWS,
        BHI=BHI,
        NB=NB,
        Nc=Nc,
        GT_lo=GT_lo,
        GT_hi=GT_hi,
        DLO=[int(d) for d in DLO],
        DHI=[int(d) for d in DHI],
        off_lo=[int(o) for o in off_lo],
        off_hi=[int(o) for o in off_hi],
    )
    orders = [co["order"] for co in cores]
    return in_maps, params, orders


def build_program(cfg, params):
    IN, H, F, C = cfg["IN"], cfg["H"], cfg["F"], cfg["C"]
    HF = H * F
    KP2, TROWS, BHI, NB = (
        params["KP2"],
        params["TROWS"],
        params["BHI"],
        params["NB"],
    )
    GT_lo, GT_hi = params["GT_lo"], params["GT_hi"]
    DLO, DHI = params["DLO"], params["DHI"]
    off_lo, off_hi = params["off_lo"], params["off_hi"]
    SUPER = cfg["SUPER"]
    phase = {1: 1, 21: 2, 22: 3, 2: 4, 3: 5, 4: 6}[cfg.get("phase", 4)]
    tdt = BF16 if cfg["table_bf16"] else FP32
    slope = float(cfg["neg_slope"])
    KC = IN // P
    MT = KP2 // P
    DMAXB = max(DLO[b] + DHI[b] for b in range(NB))
    sbs = list(range(0, NB, SUPER))
    MAXG = max(
        (off_lo[min(s + SUPER, NB)] - off_lo[s])
        + (off_hi[min(s + SUPER, NB)] - off_hi[s])
        for s in sbs
    )

    nc = bacc.Bacc("TRN2", target_bir_lowering=False, debug=False, num_devices=C)
    xcT_d = nc.dram_tensor("xcT", [IN, KP2], tdt, kind="ExternalInput").ap()
    wcat_d = nc.dram_tensor("wcat", [IN, HF + H], tdt, kind="ExternalInput").ap()
    attnl_d = nc.dram_tensor("attnl", [P, HF], tdt, kind="ExternalInput").ap()
    biasm_d = nc.dram_tensor("biasm", [P, F], FP32, kind="ExternalInput").ap()
    padrow_d = nc.dram_tensor("padrow", [P, HF], tdt, kind="ExternalInput").ap()
    widx_lo_d = nc.dram_tensor(
        "widx_lo", [P, 8 * GT_lo], I16, kind="ExternalInput"
    ).ap()
    widx_hi_d = nc.dram_tensor(
        "widx_hi", [P, 8 * max(GT_hi, 2)], I16, kind="ExternalInput"
    ).ap()
    out_d = nc.dram_tensor("out", [NB * P, F], FP32, kind="ExternalOutput").ap()

    with tile.TileContext(nc) as tc:
        with (
            tc.tile_pool(name="dram", bufs=1, space="DRAM") as dpool,
            tc.tile_pool(name="const", bufs=1) as cpool,
            tc.tile_pool(name="xa", bufs=3) as xpool,
            tc.tile_pool(name="proj", bufs=4) as fpool,
            tc.tile_pool(name="psum", bufs=4, space="PSUM") as ppool,
            tc.tile_pool(name="gat", bufs=3) as gpool,
            tc.tile_pool(name="widx", bufs=3) as wpool,
            tc.tile_pool(name="blk", bufs=3) as bpool,
        ):
            feat = dpool.tile([TROWS, HF], tdt)

            # constants
            w_t = []
            for k in range(KC):
                wt = cpool.tile([P, HF + H], tdt, tag=f"w{k}")
                nc.sync.dma_start(out=wt[:], in_=wcat_d[k * P : (k + 1) * P, :])
                w_t.append(wt)
            at = cpool.tile([P, HF], tdt, tag="attnl")
            nc.sync.dma_start(out=at[:], in_=attnl_d[:])
            bi = cpool.tile([P, F], FP32, tag="biasm")
            nc.sync.dma_start(out=bi[:], in_=biasm_d[:])
            er_all = cpool.tile([P, NB * H], FP32, tag="erall")

            # pad rows -> feat[0:128] and feat[TROWS-1]
            prt = cpool.tile([P, HF], tdt, tag="padrow")
            nc.sync.dma_start(out=prt[:], in_=padrow_d[:])
            nc.sync.dma_start(out=feat[0:P, :], in_=prt[:])
            nc.sync.dma_start(out=feat[TROWS - 1 : TROWS, :], in_=prt[0:1, :])

            # projection: feat[128+m*128 ...] = xcT[:, m].T @ [W | W@attn_r]
            XB = 4
            for m0 in range(0, MT, XB):
                nb = min(XB, MT - m0)
                xts = []
                for k in range(KC):
                    xa = xpool.tile([P, XB * P], tdt, tag=f"xa{k}")
                    nc.sync.dma_start(
                        out=xa[:, : nb * P],
                        in_=xcT_d[k * P : (k + 1) * P, m0 * P : (m0 + nb) * P],
                    )
                    xts.append(xa)
                for j in range(nb):
                    m = m0 + j
                    fsb = fpool.tile([P, HF], tdt, tag="fsb")
                    if cfg.get("nomm"):
                        nc.vector.tensor_copy(
                            out=fsb[:], in_=xts[0][:, j * P : (j + 1) * P]
                        )
                        pt = None
                    else:
                        pt = ppool.tile([P, HF + H], FP32, space="PSUM")
                        for k in range(KC):
                            nc.tensor.matmul(
                                out=pt[:],
                                lhsT=xts[k][:, j * P : (j + 1) * P],
                                rhs=w_t[k][:],
                                start=(k == 0),
                                stop=(k == KC - 1),
                            )
                        nc.vector.tensor_copy(out=fsb[:], in_=pt[:, :HF])
                    nc.sync.dma_start(
                        out=feat[P + m * P : P + (m + 1) * P, :], in_=fsb[:]
                    )
                    if m < NB and pt is not None:
                        nc.vector.tensor_copy(
                            out=er_all[:, m * H : (m + 1) * H],
                            in_=pt[:, HF : HF + H],
                        )

            # edge phase
            for s0 in (sbs if phase >= 2 else []):  # noqa
                s1 = min(s0 + SUPER, NB)
                glo = off_lo[s1] - off_lo[s0]
                ghi = off_hi[s1] - off_hi[s0]
                clo = off_lo[s0]
                chi = off_hi[s0]
                wtl = wpool.tile([P, 8 * MAXG], I16, tag="widx")
                nc.sync.dma_start(
                    out=wtl[:, : 8 * glo],
                    in_=widx_lo_d[:, 8 * clo : 8 * (clo + glo)],
                )
                if ghi > 0:
                    nc.sync.dma_start(
                        out=wtl[:, 8 * glo : 8 * (glo + ghi)],
                        in_=widx_hi_d[:, 8 * chi : 8 * (chi + ghi)],
                    )
                G = gpool.tile([P, MAXG * HF], tdt, tag="G")
                nc.gpsimd.dma_gather(
                    G[:, : glo * HF].rearrange("p (g e) -> p g e", e=HF),
                    feat[0:LOW, :] if TROWS > LOW else feat[:],
                    wtl[:, : 8 * glo],
                    P * glo,
                    P * glo,
                    HF,
                    single_packet=False,
                )
                if ghi > 0:
                    nc.gpsimd.dma_gather(
                        G[:, glo * HF : (glo + ghi) * HF].rearrange(
                            "p (g e) -> p g e", e=HF
                        ),
                        feat[BHI:TROWS, :],
                        wtl[:, 8 * glo : 8 * (glo + ghi)],
                        P * ghi,
                        P * ghi,
                        HF,
                        single_packet=False,
                    )
                for b in range(s0, s1):
                    DL, DH = DLO[b], DHI[b]
                    D = DL + DH
                    pieces = [(G[:, (off_lo[b] - clo) * HF :], DL, 0)]
                    if DH > 0:
                        pieces.append(
                            (G[:, (glo + off_hi[b] - chi) * HF :], DH, DL)
                        )
                    elp = bpool.tile([P, DMAXB * HF], tdt, tag="elp")
                    sc = bpool.tile([P, DMAXB * H], FP32, tag="sc")
                    for gsl, dn, d0 in (pieces if phase >= 5 else []):
                        # el = sum_f feat[src]*attn_l
                        nc.vector.tensor_tensor(
                            out=elp[:, d0 * HF : (d0 + dn) * HF].rearrange(
                                "p (g e) -> p g e", e=HF
                            ),
                            in0=gsl[:, : dn * HF].rearrange(
                                "p (g e) -> p g e", e=HF
                            ),
                            in1=at[:]
                            .rearrange("p (o e) -> p o e", o=1)
                            .to_broadcast([P, dn, HF]),
                            op=mybir.AluOpType.mult,
                        )
                        nc.vector.reduce_sum(
                            out=sc[:, d0 * H : (d0 + dn) * H].rearrange(
                                "p (g h) -> p g h", h=H
                            ),
                            in_=elp[:, d0 * HF : (d0 + dn) * HF].rearrange(
                                "p (g h f) -> p g h f", h=H, f=F
                            ),
                            axis=mybir.AxisListType.X,
                        )
                    # scores = leaky(el + er); ex = exp
                    if phase < 5:
                        o_t = bpool.tile([P, F], FP32, tag="o")
                        if phase == 2:
                            nc.vector.tensor_copy(
                                out=o_t[:], in_=pieces[0][0][:, :F]
                            )
                            nc.sync.dma_start(
                                out=out_d[b * P : (b + 1) * P, :], in_=o_t[:]
                            )
                            continue
                        rst = bpool.tile([P, HF], FP32, tag="rst")
                        rst2 = bpool.tile([P, HF], FP32, tag="rst2")
                        for i, (gsl, dn, d0) in enumerate(pieces):
                            nc.vector.reduce_sum(
                                out=(rst if i == 0 else rst2)[:],
                                in_=gsl[:, : dn * HF].rearrange(
                                    "p (g e) -> p e g", e=HF
                                ),
                                axis=mybir.AxisListType.X,
                            )
                        if len(pieces) > 1:
                            nc.vector.tensor_add(out=rst[:], in0=rst[:], in1=rst2[:])
                        if phase == 3:
                            nc.vector.tensor_copy(out=o_t[:], in_=rst[:, :F])
                        else:
                            nc.vector.reduce_sum(
                                out=o_t[:],
                                in_=rst[:].rearrange("p (h f) -> p f h", h=H),
                                axis=mybir.AxisListType.X,
                            )
                        nc.sync.dma_start(
                            out=out_d[b * P : (b + 1) * P, :], in_=o_t[:]
                        )
                        continue
                    nc.vector.tensor_tensor(
                        out=sc[:, : D * H].rearrange("p (g h) -> p g h", h=H),
                        in0=sc[:, : D * H].rearrange("p (g h) -> p g h", h=H),
                        in1=er_all[:, b * H : (b + 1) * H]
                        .rearrange("p (o h) -> p o h", o=1)
                        .to_broadcast([P, D, H]),
                        op=mybir.AluOpType.add,
                    )
                    nc.vector.scalar_tensor_tensor(
                        out=sc[:, : D * H],
                        in0=sc[:, : D * H],
                        scalar=slope,
                        in1=sc[:, : D * H],
                        op0=mybir.AluOpType.mult,
                        op1=mybir.AluOpType.max,
                    )
                    ex = bpool.tile([P, DMAXB * H], tdt, tag="ex")
                    nc.scalar.activation(
                        out=ex[:, : D * H],
                        in_=sc[:, : D * H],
                        func=mybir.ActivationFunctionType.Exp,
                    )
                    # s = sum_deg ex ; r = 1/max(s,eps)
                    s_t = bpool.tile([P, H], FP32, tag="s")
                    nc.vector.reduce_sum(
                        out=s_t[:],
                        in_=ex[:, : D * H].rearrange("p (g h) -> p h g", h=H),
                        axis=mybir.AxisListType.X,
                    )
                    r_t = bpool.tile([P, H], FP32, tag="r")
                    nc.vector.scalar_tensor_tensor(
                        out=r_t[:],
                        in0=s_t[:],
                        scalar=1e-9,
                        in1=s_t[:],
                        op0=mybir.AluOpType.max,
                        op1=mybir.AluOpType.bypass,
                    )
                    nc.vector.reciprocal(out=r_t[:], in_=r_t[:])
                    # weighted messages (in place on G) + aggregation
                    rst = bpool.tile([P, HF], FP32, tag="rst")
                    rst2 = bpool.tile([P, HF], FP32, tag="rst2")
                    if phase < 6:
                        for i, (gsl, dn, d0) in enumerate(pieces):
                            nc.vector.reduce_sum(
                                out=(rst if i == 0 else rst2)[:],
                                in_=gsl[:, : dn * HF].rearrange(
                                    "p (g e) -> p e g", e=HF
                                ),
                                axis=mybir.AxisListType.X,
                            )
                        if len(pieces) > 1:
                            nc.vector.tensor_add(out=rst[:], in0=rst[:], in1=rst2[:])
                        o_t = bpool.tile([P, F], FP32, tag="o")
                        nc.vector.reduce_sum(
                            out=o_t[:],
                            in_=rst[:].rearrange("p (h f) -> p f h", h=H),
                            axis=mybir.AxisListType.X,
                        )
                        nc.sync.dma_start(
                            out=out_d[b * P : (b + 1) * P, :], in_=o_t[:]
                        )
                        continue
                    for i, (gsl, dn, d0) in enumerate(pieces):
                        nc.vector.tensor_tensor(
                            out=gsl[:, : dn * HF].rearrange(
                                "p (g h f) -> p g h f", h=H, f=F
                            ),
                            in0=gsl[:, : dn * HF].rearrange(
                                "p (g h f) -> p g h f", h=H, f=F
                            ),
                            in1=ex[:, d0 * H : (d0 + dn) * H]
                            .rearrange("p (g h o) -> p g h o", h=H, o=1)
                            .to_broadcast([P, dn, H, F]),
                            op=mybir.AluOpType.mult,
                        )
                        nc.vector.reduce_sum(
                            out=(rst if i == 0 else rst2)[:],
                            in_=gsl[:, : dn * HF].rearrange(
                                "p (g e) -> p e g", e=HF
                            ),
                            axis=mybir.AxisListType.X,
                        )
                    if len(pieces) > 1:
                        nc.vector.tensor_add(out=rst[:], in0=rst[:], in1=rst2[:])
                    # normalize, head-mean, bias
                    nc.vector.tensor_tensor(
                        out=rst[:].rearrange("p (h f) -> p h f", h=H),
                        in0=rst[:].rearrange("p (h f) -> p h f", h=H),
                        in1=r_t[:]
                        .rearrange("p (h o) -> p h o", o=1)
                        .to_broadcast([P, H, F]),
                        op=mybir.AluOpType.mult,
                    )
                    o_t = bpool.tile([P, F], FP32, tag="o")
                    nc.vector.reduce_sum(
                        out=o_t[:],
                        in_=rst[:].rearrange("p (h f) -> p f h", h=H),
                        axis=mybir.AxisListType.X,
                    )
                    nc.vector.scalar_tensor_tensor(
                        out=o_t[:],
                        in0=o_t[:],
                        scalar=1.0 / H,
                        in1=bi[:],
                        op0=mybir.AluOpType.mult,
                        op1=mybir.AluOpType.add,
                    )
                    nc.sync.dma_start(out=out_d[b * P : (b + 1) * P, :], in_=o_t[:])

    nc.compile()
    return nc


def _run(inputs, cfg, trace=False):
    in_maps, params, orders = preprocess(
        inputs["x"],
        inputs["W"],
        inputs["attn_l"],
        inputs["attn_r"],
        inputs["bias"],
        inputs["src"],
        inputs["dst"],
        cfg,
    )
    nc = build_program(cfg, params)
    res = run_bass_kernel_spmd(nc, in_maps, list(range(cfg["C"])), trace=trace)
    N, C, F = cfg["N"], cfg["C"], cfg["F"]
    Nc = N // C
    out = np.empty((N, F), np.float32)
    for c in range(C):
        oc = res.results[c]["out"]
        out[orders[c]] = oc[:Nc]
    return out, res


def kernel(**inputs):
    out, _ = _run(inputs, DEFAULT_CFG, trace=False)
    return out
